# revision 20
# baseline (speedup 1.0000x reference)
"""Trainium2 Bass kernel: single-head attention encoder block (bf16 compute).

Problem: x[4, 2048, 1024]; q/k/v projections, softmax attention, output
projection, layernorm.  8 NeuronCores, SPMD.

Sharding: core c handles batch b = c // 2 and query-half h = c % 2.
Each core receives its batch's x ROTATED along the sequence axis so that
the core's 1024 query rows always occupy rows 0:1024 (attention is
permutation-invariant over keys as long as K and V share an ordering).

All matmul operands are bf16 (PSUM accumulation is fp32; LN statistics
and the final output are fp32).  bf16 operands run the PE at the same
1 cycle/row as fp32r but halve SBUF/DMA footprint, which lets every
weight and x tile be prefetched long before its phase — the PE never
waits on DMA mid-kernel.  The host uploads x twice (row layout and
pre-transposed), removing the on-device transpose phase entirely.

Per-core dataflow (value path uses ctx = A @ (x @ Wv) = (A @ x) @ Wv):

  K^T   = Wk^T @ x          (lhsT=Wk tiles, rhs=x^T tiles from DRAM)
  Q^T   = Wq^T @ x[:1024]
  S^T   = K Q^T             ([s partition, q free] -> exp via ACT, bf16)
  den   = ones^T @ colsum   (PE broadcast-matmul, interleaved into C1)
  Z^T   = x^T @ exp(S^T)    (lhsT = x row tiles, resident in SBUF)
  ctxT  = (Wv^T @ Z^T)/den  (normalization fused into the PSUM drain)
  h     = ctx @ Wo          (fp32)
  out   = layernorm(h) * gamma + beta   (fused DVE + Pool ops)

Phase order S(0) S(1) C1 C2(0) [C2(1)||O(0)] O(1) keeps every matmul's
operands one full phase ahead of their use, so the only PE stalls are
the initial weight/x DMA (~9 us) and drain latency at two boundaries.
"""

import hashlib
from contextlib import ExitStack

import numpy as np
import ml_dtypes

import concourse.bass as bass
import concourse.tile as tile
from concourse import bacc, mybir
from concourse.bass_utils import run_bass_kernel_spmd

F32 = mybir.dt.float32
F32R = mybir.dt.float32r
BF16 = mybir.dt.bfloat16
AF = mybir.ActivationFunctionType
OP = mybir.AluOpType

B = 4
S = 2048
D = 1024
NQ = 1024  # queries per core
P = 128
DT = D // P   # 8 d-tiles
ST = S // P   # 16 s-tiles
KTN = D // P  # 8 k-tiles
QTN = NQ // P  # 8 q-tiles
NC = 512      # matmul free-dim chunk (one fp32 PSUM bank)
SCN = S // NC   # 4 s-chunks
QCN = NQ // NC  # 2 q-chunks
DCN = D // NC   # 2 d-chunks
N_CORES = 8
SCALE = 1.0 / np.sqrt(np.float32(D))  # 1/32
LN_EPS = 1e-5
N_WARM = 16  # PE warmup matmuls covering the initial DMA window

BFNP = ml_dtypes.bfloat16

# The libneuronxla NEFF cache keys on an HLO hash that does not always
# capture the embedded bass program, so two kernel versions with the same
# I/O signature can silently reuse each other's compiled NEFF.  Naming
# the output tensor after a hash of this file makes every source change
# produce a distinct HLO and defeats the stale cache.
with open(__file__, "rb") as _f:
    _SRC_TAG = hashlib.sha256(_f.read()).hexdigest()[:10]
OUT_NAME = f"out_{_SRC_TAG}"


def _keepalive(nc, tc, aps, out):
    """Read one column of each AP and DMA to out so bacc keeps the work."""
    kp = tc.alloc_tile_pool(name="keep", bufs=1, side="left")
    kt = kp.tile([P, max(len(aps), 1)], F32, tag="keep", name="keept")
    for i, ap in enumerate(aps):
        nc.vector.tensor_copy(kt[:, i:i + 1], ap[:, 0:1])
    nc.sync.dma_start(out[0:P, 0:max(len(aps), 1)], kt[:])
    kp.release()


def _emit(ctx: ExitStack, tc: tile.TileContext, io: dict, upto: str = "full"):
    nc = tc.nc
    xb = io["xb"]          # [S, D] bf16 (rows; C1 lhsT)
    xt = io["xt"]          # [D, S] bf16 (pre-transposed; K/Q rhs)
    wq = io["wq"]          # [D, D] bf16
    wk = io["wk"]
    wv = io["wv"]
    wo = io["wo"]
    gamma_b = io["gamma_b"]  # [P, D] f32
    beta_b = io["beta_b"]
    out = io["out"]        # [NQ, D] f32

    const = ctx.enter_context(tc.tile_pool(name="const", bufs=1, side="left"))
    ones_f = const.tile([P, P], F32, tag="ones_f")
    nc.vector.memset(ones_f[:], 1.0)
    ones = const.tile([P, P], F32R, tag="ones")
    nc.vector.tensor_copy(ones[:], ones_f[:])
    recip = const.tile([P, NQ], F32, tag="recip")
    eps_sb = const.tile([P, 1], F32, tag="eps")
    nc.vector.memset(eps_sb[:], LN_EPS)

    # PSUM: 8 banks = 6 matmul + 2 denominator.
    ps_mm = ctx.enter_context(tc.tile_pool(name="ps_mm", bufs=6, space="PSUM"))
    ps_den = ctx.enter_context(tc.tile_pool(name="ps_den", bufs=2, space="PSUM"))

    # Left stack (LIFO): xtb | wq | wk, then (after release) at | den.
    xtb_pool = tc.alloc_tile_pool(name="xtb", bufs=1, side="left")
    xtb = [xtb_pool.tile([P, S], BF16, tag=f"xtb{d}", name=f"xtb{d}") for d in range(DT)]
    wq_pool = tc.alloc_tile_pool(name="wqp", bufs=1, side="left")
    wq_sb = [wq_pool.tile([P, D], BF16, tag=f"wqr{d}", name=f"wqr{d}") for d in range(DT)]
    wk_pool = tc.alloc_tile_pool(name="wkp", bufs=1, side="left")
    wk_sb = [wk_pool.tile([P, D], BF16, tag=f"wkr{d}", name=f"wkr{d}") for d in range(DT)]

    # Right stack: wv | wo | gb | xrow | kt | qt, then ctxT | zt | h | u | o.
    wv_pool = tc.alloc_tile_pool(name="wvp", bufs=1, side="right")
    wv_sb = [wv_pool.tile([P, D], BF16, tag=f"wv{d}", name=f"wv{d}") for d in range(DT)]
    wo_pool = tc.alloc_tile_pool(name="wop", bufs=1, side="right")
    wo_sb = [wo_pool.tile([P, D], BF16, tag=f"wo{v}", name=f"wo{v}") for v in range(DT)]
    gb_pool = tc.alloc_tile_pool(name="gbp", bufs=1, side="right")
    gamma_sb = gb_pool.tile([P, D], F32, tag="gamma", name="gamma_sb")
    beta_sb = gb_pool.tile([P, D], F32, tag="beta", name="beta_sb")
    xrow_pool = tc.alloc_tile_pool(name="xrow", bufs=1, side="right")
    xrow = [xrow_pool.tile([P, D], BF16, tag=f"xr{st}", name=f"xr{st}") for st in range(ST)]
    kt_pool = tc.alloc_tile_pool(name="ktp", bufs=1, side="right")
    kt_sb = [kt_pool.tile([P, S], BF16, tag=f"kt{k}", name=f"kt{k}") for k in range(KTN)]
    qt_pool = tc.alloc_tile_pool(name="qtp", bufs=1, side="right")
    qt_sb = [qt_pool.tile([P, NQ], BF16, tag=f"qt{k}", name=f"qt{k}") for k in range(KTN)]

    # ---- DMA: K-phase operands split across the three queues so the
    # serial per-queue dispatch cost (~1.3us each) overlaps.  Everything
    # else follows on SP/Pool, which have nothing  to dispatch mid-kernel.
    for d in range(DT):
        nc.scalar.dma_start(wk_sb[d][:], wk[d * P:(d + 1) * P, :])     # ACT
        nc.sync.dma_start(xtb[d][:, 0:NC], xt[d * P:(d + 1) * P, 0:NC])  # SP
    for d in range(DT):
        nc.sync.dma_start(xtb[d][:, NC:S], xt[d * P:(d + 1) * P, NC:S])
    for d in range(DT):
        nc.sync.dma_start(wq_sb[d][:], wq[d * P:(d + 1) * P, :])
    for d in range(DT):
        nc.sync.dma_start(wv_sb[d][:], wv[d * P:(d + 1) * P, :])
    for v in range(DT):
        nc.sync.dma_start(wo_sb[v][:], wo[v * P:(v + 1) * P, :])
    nc.sync.dma_start(gamma_sb[:], gamma_b[:])
    nc.sync.dma_start(beta_sb[:], beta_b[:])
    for st in range(ST):
        nc.sync.dma_start(xrow[st][:], xb[st * P:(st + 1) * P, :])

    # PE warmup: dependency-free dummy matmuls cover the initial DMA
    # window and ramp the tensor-engine p-state before real work.
    for i in range(N_WARM):
        wm = ps_den.tile([P, NC], F32, tag="den", name=f"warm{i}")
        nc.tensor.matmul(wm[:, 0:P], ones[:], ones[:], start=True, stop=True)

    # ---- Phase K: K^T = Wk^T @ x  ([k, s]) ----
    for sc in range(SCN):
        for k in range(KTN):
            ps = ps_mm.tile([P, NC], F32, tag="mm", name=f"psK{k}_{sc}")
            for d in range(DT):
                nc.tensor.matmul(
                    ps[:],
                    wk_sb[d][:, k * P:(k + 1) * P],
                    xtb[d][:, sc * NC:(sc + 1) * NC],
                    start=(d == 0),
                    stop=(d == DT - 1),
                )
            if k % 2 == 0:
                nc.vector.tensor_copy(kt_sb[k][:, sc * NC:(sc + 1) * NC], ps[:])
            else:
                nc.scalar.copy(kt_sb[k][:, sc * NC:(sc + 1) * NC], ps[:])
    wk_pool.release()

    if upto == "K":
        _keepalive(nc, tc, [t[:, 0:1] for t in kt_sb] + [t[:, 0:1] for t in xtb], out)
        for p in [qt_pool, kt_pool, xrow_pool, gb_pool, wo_pool, wv_pool,
                  wq_pool, xtb_pool]:
            p.release()
        return

    # ---- Phase Q: Q^T = Wq^T @ x[:, :NQ]  ([k, q]) ----
    for k in range(KTN):
        for qc in range(QCN):
            ps = ps_mm.tile([P, NC], F32, tag="mm", name=f"psQ{k}_{qc}")
            for d in range(DT):
                nc.tensor.matmul(
                    ps[:],
                    wq_sb[d][:, k * P:(k + 1) * P],
                    xtb[d][:, qc * NC:(qc + 1) * NC],
                    start=(d == 0),
                    stop=(d == DT - 1),
                )
            if k % 2 == 0:
                nc.vector.tensor_copy(qt_sb[k][:, qc * NC:(qc + 1) * NC], ps[:])
            else:
                nc.scalar.copy(qt_sb[k][:, qc * NC:(qc + 1) * NC], ps[:])
    wq_pool.release()
    xtb_pool.release()

    if upto == "Q":
        _keepalive(nc, tc, [t[:, 0:1] for t in kt_sb] + [t[:, 0:1] for t in qt_sb], out)
        for p in [qt_pool, kt_pool, xrow_pool, gb_pool, wo_pool, wv_pool]:
            p.release()
        return

    # ---- Phase S: scores^T -> exp (UNNORMALIZED bf16), denominator sums ----
    at_pool = tc.alloc_tile_pool(name="atp", bufs=1, side="left")
    at_sb = [at_pool.tile([P, NQ], BF16, tag=f"at{st}", name=f"at{st}") for st in range(ST)]
    den_pool = tc.alloc_tile_pool(name="denp", bufs=2, side="left")
    dsb = [den_pool.tile([P, NC], F32, tag="densb", name=f"densb{qc}") for qc in range(QCN)]
    dsr = [den_pool.tile([P, NC], F32R, tag="densr", name=f"densr{qc}") for qc in range(QCN)]
    for qc in range(QCN):
        nc.vector.memset(dsb[qc][:], 0.0)
    for qc in range(QCN):
        for st in range(ST):
            ps = ps_mm.tile([P, NC], F32, tag="mm", name=f"psS{qc}_{st}")
            for k in range(KTN):
                nc.tensor.matmul(
                    ps[:],
                    kt_sb[k][:, st * P:(st + 1) * P],
                    qt_sb[k][:, qc * NC:(qc + 1) * NC],
                    start=(k == 0),
                    stop=(k == KTN - 1),
                )
            # attn = exp(scores / sqrt(dk)); max-subtraction unnecessary here
            # (scores are O(1) by construction) and softmax is shift-invariant.
            nc.scalar.activation(
                at_sb[st][:, qc * NC:(qc + 1) * NC], ps[:], AF.Exp, scale=float(SCALE)
            )
            nc.vector.tensor_tensor(
                dsb[qc][:], dsb[qc][:], at_sb[st][:, qc * NC:(qc + 1) * NC], OP.add
            )
    qt_pool.release()
    kt_pool.release()
    for qc in range(QCN):
        nc.vector.tensor_copy(dsr[qc][:], dsb[qc][:])

    if upto == "S":
        _keepalive(nc, tc, [t[:, 0:1] for t in at_sb]
                   + [dsr[0][:, 0:1], dsr[1][:, 0:1]], out)
        den_pool.release()
        at_pool.release()
        for p in [xrow_pool, gb_pool, wo_pool, wv_pool]:
            p.release()
        return

    # ---- Phase C1: Z^T = x^T @ exp(S^T)  ([d, q]; x row tiles resident) ----
    # den broadcast-matmuls (column sums over all 128 partitions via
    # ones^T @ dsr) slot between the first C1 groups: operands are long
    # ready, so they cost 2 x 213ns of PE with zero stall.
    ctxT_pool = tc.alloc_tile_pool(name="ctxTp", bufs=1, side="right")
    ctxT = [ctxT_pool.tile([P, NQ], BF16, tag=f"cxT{v}", name=f"cxT{v}") for v in range(DT)]
    zt_pool = tc.alloc_tile_pool(name="ztp", bufs=1, side="right")
    zt_sb = [zt_pool.tile([P, NQ], BF16, tag=f"zt{d}", name=f"zt{d}") for d in range(DT)]
    for d in range(DT):
        for qc in range(QCN):
            ps = ps_mm.tile([P, NC], F32, tag="mm", name=f"psZ{d}_{qc}")
            for st in range(ST):
                nc.tensor.matmul(
                    ps[:],
                    xrow[st][:, d * P:(d + 1) * P],
                    at_sb[st][:, qc * NC:(qc + 1) * NC],
                    start=(st == 0),
                    stop=(st == ST - 1),
                )
            if d % 2 == 0:
                nc.vector.tensor_copy(zt_sb[d][:, qc * NC:(qc + 1) * NC], ps[:])
            else:
                nc.scalar.copy(zt_sb[d][:, qc * NC:(qc + 1) * NC], ps[:])
        if d < QCN:
            dps = ps_den.tile([P, NC], F32, tag="den", name=f"dps{d}")
            nc.tensor.matmul(dps[:], ones[:], dsr[d][:], start=True, stop=True)
            nc.vector.reciprocal(recip[:, d * NC:(d + 1) * NC], dps[:])
    den_pool.release()
    at_pool.release()

    if upto == "C1":
        _keepalive(nc, tc, [t[:, 0:1] for t in zt_sb] + [recip[:, 0:1]], out)
        for p in [zt_pool, ctxT_pool, xrow_pool, gb_pool, wo_pool, wv_pool]:
            p.release()
        return

    # ---- Phase C2 + O, interleaved per q-chunk ----
    # C2: ctxT = (Wv^T @ Z^T) * 1/den.  O: h = ctx @ Wo -> layernorm -> out.
    # Order: C2(qc0) | [C2(qc1) group i ; O(qc0) group i] | O(qc1), so every
    # O group's ctxT columns were drained a full phase earlier.
    h_pool = tc.alloc_tile_pool(name="hp", bufs=2, side="right")
    u_pool = tc.alloc_tile_pool(name="up", bufs=2, side="right")
    o_pool = tc.alloc_tile_pool(name="op", bufs=2, side="right")
    sq_pool = tc.alloc_tile_pool(name="sqp", bufs=2, side="right")
    stat_pool = tc.alloc_tile_pool(name="statp", bufs=4, side="right")
    h_cur = [None]

    def c2_group(vt, qc):
        ps = ps_mm.tile([P, NC], F32, tag="mm", name=f"psC{vt}_{qc}")
        for d in range(DT):
            nc.tensor.matmul(
                ps[:],
                wv_sb[d][:, vt * P:(vt + 1) * P],
                zt_sb[d][:, qc * NC:(qc + 1) * NC],
                start=(d == 0),
                stop=(d == DT - 1),
            )
        # normalization fused into the PSUM drain
        nc.vector.tensor_tensor(
            ctxT[vt][:, qc * NC:(qc + 1) * NC], ps[:],
            recip[:, qc * NC:(qc + 1) * NC], OP.mult,
        )

    ln_state = {}

    def o_group(qt, dc):
        if dc == 0:
            h_cur[0] = h_pool.tile([P, D], F32, tag="h", name=f"h{qt}")
            ln_state["acc"] = stat_pool.tile([P, 4], F32, tag="acc", name=f"acc{qt}")
        h = h_cur[0]
        acc = ln_state["acc"]  # [:,0:2] = sum(h) halves, [:,2:4] = sum(h^2)
        ps = ps_mm.tile([P, NC], F32, tag="mm", name=f"psO{qt}_{dc}")
        for v in range(DT):
            nc.tensor.matmul(
                ps[:],
                ctxT[v][:, qt * P:(qt + 1) * P],
                wo_sb[v][:, dc * NC:(dc + 1) * NC],
                start=(v == 0),
                stop=(v == DT - 1),
            )
        # LN statistics come free from the ACT accumulators: the Square
        # pass gives sum(h^2), the drain itself gives sum(h).  Both read
        # PSUM directly; the dc=0 costs hide under dc=1's matmuls.
        sq = sq_pool.tile([P, NC], F32, tag="sq", name=f"sq{qt}_{dc}")
        nc.scalar.activation(sq[:], ps[:], AF.Square,
                             accum_out=acc[:, 2 + dc:3 + dc])
        nc.scalar.activation(h[:, dc * NC:(dc + 1) * NC], ps[:], AF.Copy,
                             accum_out=acc[:, dc:dc + 1])
        if dc == DCN - 1:
            # mu = (m0+m1)/D; var = (q0+q1)/D - mu^2  (mu << sigma here,
            # so the subtraction loses nothing).  All [P,1] ops.
            mv = stat_pool.tile([P, 2], F32, tag="mv", name=f"mv{qt}")
            nc.vector.tensor_tensor(mv[:, 0:1], acc[:, 0:1], acc[:, 1:2], OP.add)
            nc.vector.tensor_scalar(out=mv[:, 0:1], in0=mv[:, 0:1],
                                    scalar1=1.0 / D, scalar2=None, op0=OP.mult)
            nc.vector.tensor_tensor(mv[:, 1:2], acc[:, 2:3], acc[:, 3:4], OP.add)
            musq = stat_pool.tile([P, 1], F32, tag="musq", name=f"musq{qt}")
            nc.vector.tensor_tensor(musq[:], mv[:, 0:1], mv[:, 0:1], OP.mult)
            nc.vector.tensor_scalar(out=mv[:, 1:2], in0=mv[:, 1:2],
                                    scalar1=1.0 / D, scalar2=musq[:],
                                    op0=OP.mult, op1=OP.subtract)
            rstd = stat_pool.tile([P, 1], F32, tag="rstd", name=f"rstd{qt}")
            nc.scalar.activation(rstd[:], mv[:, 1:2], AF.Sqrt, bias=eps_sb[:], scale=1.0)
            nc.vector.reciprocal(rstd[:], rstd[:])
            # Affine in chunks so the post-matmul serial tail is one
            # chunk of each stage; stores overlap later chunks.  The
            # final tile uses quarter chunks to shorten the kernel tail.
            u = u_pool.tile([P, D], F32, tag="u", name=f"u{qt}")
            o = o_pool.tile([P, D], F32, tag="o", name=f"o{qt}")
            nchunk = 2
            cw = D // nchunk
            for i in range(nchunk):
                sl = slice(i * cw, (i + 1) * cw)
                nc.vector.scalar_tensor_tensor(
                    u[:, sl], h[:, sl], mv[:, 0:1], gamma_sb[:, sl],
                    OP.subtract, OP.mult)
                nc.vector.scalar_tensor_tensor(
                    o[:, sl], u[:, sl], rstd[:], beta_sb[:, sl],
                    OP.mult, OP.add)
                nc.sync.dma_start(out[qt * P:(qt + 1) * P, sl], o[:, sl])

    for vt in range(DT):
        c2_group(vt, 0)
    if upto == "C2":
        for vt in range(DT):
            c2_group(vt, 1)
        _keepalive(nc, tc, [t[:, 0:1] for t in ctxT], out)
        for p in [stat_pool, sq_pool, o_pool, u_pool, h_pool, zt_pool, ctxT_pool,
                  xrow_pool, gb_pool, wo_pool, wv_pool]:
            p.release()
        return
    for i in range(DT):
        c2_group(i, 1)
        o_group(i // DCN, i % DCN)
    for qt in range(QTN // 2, QTN):
        for dc in range(DCN):
            o_group(qt, dc)

    for p in [stat_pool, sq_pool, o_pool, u_pool, h_pool, zt_pool, ctxT_pool,
              xrow_pool, gb_pool, wo_pool, wv_pool]:
        p.release()


_PROGS: dict = {}


def _build_program(n_iters: int = 1, upto: str = "full"):
    key = (n_iters, upto)
    if key not in _PROGS:
        nc = bacc.Bacc(
            "TRN2",
            target_bir_lowering=False,
            debug=False,
            enable_asserts=False,
            num_devices=N_CORES,
        )
        io = {
            "xb": nc.dram_tensor("xb", [S, D], BF16, kind="ExternalInput").ap(),
            "xt": nc.dram_tensor("xt", [D, S], BF16, kind="ExternalInput").ap(),
            "wq": nc.dram_tensor("wq", [D, D], BF16, kind="ExternalInput").ap(),
            "wk": nc.dram_tensor("wk", [D, D], BF16, kind="ExternalInput").ap(),
            "wv": nc.dram_tensor("wv", [D, D], BF16, kind="ExternalInput").ap(),
            "wo": nc.dram_tensor("wo", [D, D], BF16, kind="ExternalInput").ap(),
            "gamma_b": nc.dram_tensor("gamma_b", [P, D], F32, kind="ExternalInput").ap(),
            "beta_b": nc.dram_tensor("beta_b", [P, D], F32, kind="ExternalInput").ap(),
            "out": nc.dram_tensor(OUT_NAME, [NQ, D], F32, kind="ExternalOutput").ap(),
        }
        with tile.TileContext(nc) as tc:
            for _ in range(n_iters):
                with ExitStack() as ctx:
                    _emit(ctx, tc, io, upto)
        nc.compile()
        _PROGS[key] = nc
    return _PROGS[key]


LAST_RESULTS = None


def _core_inputs(x, Wq, Wk, Wv, Wo, ln2_gamma, ln2_beta):
    """Host-side prep: bf16 casts, per-core roll, both x layouts."""
    xq = np.asarray(x, dtype=np.float32).astype(BFNP)
    w = {
        n: np.ascontiguousarray(np.asarray(a, dtype=np.float32).astype(BFNP))
        for n, a in [("wq", Wq), ("wk", Wk), ("wv", Wv), ("wo", Wo)]
    }
    gamma_b = np.ascontiguousarray(
        np.broadcast_to(np.asarray(ln2_gamma, dtype=np.float32), (P, D)))
    beta_b = np.ascontiguousarray(
        np.broadcast_to(np.asarray(ln2_beta, dtype=np.float32), (P, D)))
    in_maps = []
    for c in range(N_CORES):
        b, h = c // 2, c % 2
        # Rotate so this core's query rows are rows 0:NQ.
        xb = np.ascontiguousarray(np.roll(xq[b], -h * NQ, axis=0))
        in_maps.append({
            "xb": xb,
            "xt": np.ascontiguousarray(xb.T),
            **w,
            "gamma_b": gamma_b,
            "beta_b": beta_b,
        })
    return in_maps


def kernel(x, Wq, Wk, Wv, Wo, ln2_gamma, ln2_beta):
    global LAST_RESULTS
    in_maps = _core_inputs(x, Wq, Wk, Wv, Wo, ln2_gamma, ln2_beta)
    nc = _build_program()
    res = run_bass_kernel_spmd(nc, in_maps, list(range(N_CORES)))
    LAST_RESULTS = res
    out = np.empty((B, S, D), dtype=np.float32)
    for c in range(N_CORES):
        b, h = c // 2, c % 2
        out[b, h * NQ:(h + 1) * NQ] = res.results[c][OUT_NAME]
    return out
